# revision 18
# baseline (speedup 1.0000x reference)
"""Bond-message embedding kernel for TRN2 (8 NeuronCores, SPMD).

Computes out[e, :] = concat(V[src[e]], E[e]) @ W.T + b for 800k edges.

Sharding: edges split evenly across the 8 cores (data-parallel over the
edge dim); V, W, b replicated; no cross-core comm.

Shipped design ("v4", paired transpose-gather, bf16 end-to-end):
  * All device tensors are bf16 (rel-err budget 2e-2; bf16 keeps it
    ~3e-3). PSUM accumulates in f32.
  * The SWDGE gather on real HW is charged per DESCRIPTOR, so the host
    sorts each core's edges by src node; in unique-rank space every
    adjacent pair of sorted edges has rank-delta d in {0, 1}, and a
    per-core table  Vu2[k] = [V[uniq[k]], V[uniq[k]], V[uniq[k+1]]]
    (row pitch 384 elements) lets ONE 512B transpose-gather descriptor
    fetch BOTH edges of a pair: d=0 pairs read columns 0:256, d=1 pairs
    read columns 128:384 (elem_step=384). This halves descriptor count
    vs a per-edge gather (measured ~2.1x faster end-to-end).
  * dma_gather(transpose=True) writes the rows FEATURE-major so the
    matmuls consume the gather output directly as lhsT - no transposes.
  * E is host-pre-transposed to feature-major [65, n_slots] with a 65th
    row of ones; W2tb = [W.T rows 128:192; b] so the bias falls out of
    the accumulating matmul (exact f32 bias in PSUM, no extra vector op).
  * Chunks (2048 edges = 1024 pairs) are homogeneous in (d, rank-half)
    -- rank halves keep int16 indices valid; the 4-group geometry adapts
    to the input and compiles on first call.
  * Output slots are permuted so each partition stores 16 contiguous
    HBM rows (8KB descriptors); PSUM evacuation (f32 -> bf16 cast)
    alternates between the DVE and Activation engines.
  * The host undoes all permutations and converts bf16 -> f32.
  * Single SWDGE queue: multi-queue (num_swdge_queues > 1) produces
    wrong results on this HW/runtime path (verified empirically).

Per-chunk device work: 1 transpose-gather (1024 x 512B descriptors),
1 E load ([65, 2048] bf16), 16 pairs of accumulating bf16 matmuls
([128|65, 128] x [.., 256] -> PSUM), 8 PSUM->SBUF cast-copies, 1 output
store ([128, 16*256] bf16).
"""

import numpy as np
import ml_dtypes

import concourse.bacc as bacc
import concourse.mybir as mybir
import concourse.tile as tile
from concourse.bass_utils import run_bass_kernel_spmd

F32 = mybir.dt.float32
BF16 = mybir.dt.bfloat16
I16 = mybir.dt.int16
NP_BF16 = ml_dtypes.bfloat16

N_CORES = 8
N_NODES = 50000
ATOM = 128
BOND = 64
MSG = 256
N_EDGES = 800000
SPLIT = 25000          # int16-safe index boundary (both halves < 32768)

PER_CORE_RAW = N_EDGES // N_CORES   # 100000
P = 128
K_BLK = 16                          # 128-edge blocks per chunk
CHUNK = P * K_BLK                   # 2048 edges per chunk
LOW_CHUNKS = 25                     # 51200 low slots (expect ~50000)
HIGH_CHUNKS = 25                    # 51200 high slots (expect ~50000)
N_CHUNKS = LOW_CHUNKS + HIGH_CHUNKS
LOW_SLOTS = LOW_CHUNKS * CHUNK
HIGH_SLOTS = HIGH_CHUNKS * CHUNK
PER_CORE = N_CHUNKS * CHUNK         # 102400 device slots per core

IDX_COLS = CHUNK // 16              # 128 idx columns per chunk

# position q = j*128 + p within a chunk maps to out slot p*K_BLK + j
_Q = np.arange(CHUNK)
POS2SLOT = (_Q % P) * K_BLK + (_Q // P)          # [CHUNK]


def _emit_pipeline(nc, tc, n_chunks, low_chunks, k_blk, n_nodes, split,
                   handles, reps=1, n_queues=1, gather="tr", store=True,
                   eload="act", compute=True, evac="dve-act",
                   single_packet=False, gather_elem=ATOM):
    v_h, e_h, idx_h, w1_h, w2_h, out_h = handles
    chunk = P * k_blk
    idx_cols = chunk // 16
    g_sub = gather_elem // ATOM        # V rows per gather descriptor
    g_nidx = chunk // g_sub            # gather descriptors per chunk

    with (
        tc.tile_pool(name="const", bufs=1) as const,
        tc.tile_pool(name="chunkio", bufs=4) as chunkio,
        tc.tile_pool(name="work", bufs=3) as work,
        tc.tile_pool(name="psum", bufs=8, space="PSUM") as psum,
    ):
        # --- constants -----------------------------------------------------
        w1t = const.tile([ATOM, MSG], BF16)       # W.T rows 0:128 (atom part)
        nc.sync.dma_start(out=w1t[:], in_=w1_h[:, :])
        w2tb = const.tile([BOND + 1, MSG], BF16)  # [W.T rows 128:192; bias]
        nc.sync.dma_start(out=w2tb[:], in_=w2_h[:, :])

        # whole-core gather indices, preloaded in a few big DMAs
        idx_all = const.tile([P, n_chunks * idx_cols], I16)
        pre = min(4, n_chunks)
        nc.sync.dma_start(
            out=idx_all[:, 0:pre * idx_cols], in_=idx_h[:, 0:pre * idx_cols]
        )
        if n_chunks > pre:
            nc.sync.dma_start(
                out=idx_all[:, pre * idx_cols:],
                in_=idx_h[:, pre * idx_cols:],
            )

        o_const = None
        if not compute and store:
            o_const = const.tile([P, k_blk * MSG], BF16)
            nc.vector.memset(o_const[:], 0.25)

        # --- main loop -----------------------------------------------------
        def chunk_body():
            for c in range(n_chunks):
                c0 = c * chunk
                c1 = c0 + chunk
                v_base = (
                    v_h[0:split, :] if c < low_chunks else v_h[split:n_nodes, :]
                )

                # feature-major gathered V: [128 atom features, chunk edges]
                v_t = chunkio.tile([P, chunk], BF16, tag="vgat")
                if gather == "tr":
                    nc.gpsimd.dma_gather(
                        out_ap=v_t[:].rearrange("p (o k) -> p o k", o=g_sub),
                        in_ap=v_base,
                        idxs_ap=idx_all[
                            :, c * idx_cols:c * idx_cols + g_nidx // 16
                        ],
                        num_idxs=g_nidx,
                        num_idxs_reg=g_nidx,
                        elem_size=gather_elem,
                        transpose=True,
                        single_packet=single_packet,
                        queue_num=c % n_queues,
                    )
                elif gather == "flat":
                    nc.gpsimd.dma_gather(
                        out_ap=v_t[:].rearrange("p (k f) -> p k f", f=ATOM),
                        in_ap=v_base,
                        idxs_ap=idx_all[:, c * idx_cols:(c + 1) * idx_cols],
                        num_idxs=chunk,
                        num_idxs_reg=chunk,
                        elem_size=ATOM,
                        transpose=False,
                        single_packet=single_packet,
                        queue_num=c % n_queues,
                    )
                elif gather == "plain":
                    x0 = (c * chunk) % (n_nodes - chunk - 1)
                    nc.sync.dma_start(
                        out=v_t[:],
                        in_=v_h[x0:x0 + chunk, :].rearrange(
                            "(p k) f -> p (k f)", p=P
                        ),
                    )
                # feature-major E (+ ones row): [65, chunk edges]
                e_t = None
                if eload != "off":
                    e_t = chunkio.tile([BOND + 1, chunk], BF16, tag="ebond")
                    eng = nc.scalar if eload == "act" else nc.sync
                    eng.dma_start(out=e_t[:], in_=e_h[:, c0:c1])

                if compute:
                    o_grp = work.tile([P, k_blk * MSG], BF16, tag="ogrp")
                    for j2 in range(0, k_blk, 2):
                        o_ps = psum.tile([P, 2 * MSG], F32, tag="ops")
                        for u in range(2):
                            j = j2 + u
                            nc.tensor.matmul(
                                out=o_ps[:, u * MSG:(u + 1) * MSG],
                                lhsT=v_t[:, j * P:(j + 1) * P],
                                rhs=w1t[:],
                                start=True,
                                stop=(e_t is None),
                            )
                            if e_t is not None:
                                nc.tensor.matmul(
                                    out=o_ps[:, u * MSG:(u + 1) * MSG],
                                    lhsT=e_t[:, j * P:(j + 1) * P],
                                    rhs=w2tb[:],
                                    start=False,
                                    stop=True,
                                )
                        # PSUM -> SBUF evacuation with f32 -> bf16 cast; bias
                        # is in PSUM via the ones-row x bias-row matmul.
                        dst = o_grp[:, j2 * MSG:(j2 + 2) * MSG]
                        half = j2 // 2
                        if evac == "dve-act":
                            use = ("dve", "act")[half % 2]
                        elif evac == "spread":
                            use = ("dve", "act", "dve", "act", "pool",
                                   "dve", "act", "pool")[half % 8]
                        else:
                            use = evac
                        if use == "dve":
                            nc.vector.tensor_copy(out=dst, in_=o_ps[:])
                        elif use == "act":
                            nc.scalar.copy(out=dst, in_=o_ps[:])
                        else:
                            nc.gpsimd.tensor_copy(out=dst, in_=o_ps[:])
                else:
                    o_grp = o_const

                if store:
                    # slot s = c0 + p*k_blk + j holds position j*128+p: each
                    # partition stores k_blk contiguous HBM rows (8KB descs)
                    out_view = out_h[c0:c1, :].rearrange(
                        "(p k) m -> p k m", p=P
                    )
                    nc.sync.dma_start(out=out_view[:, :, :], in_=o_grp[:, :])

        if reps == 1:
            chunk_body()
        else:
            with tc.For_i(0, reps, 1):
                chunk_body()


def build_nc(n_chunks=N_CHUNKS, low_chunks=LOW_CHUNKS, k_blk=K_BLK,
             n_nodes=N_NODES, split=SPLIT, reps=1, n_queues=1, **probe_kw):
    chunk = P * k_blk
    per_core = n_chunks * chunk
    idx_cols = chunk // 16

    nc = bacc.Bacc(num_swdge_queues=n_queues)
    g_elem = probe_kw.get("gather_elem", ATOM)
    handles = (
        nc.declare_dram_parameter("V", [n_nodes, g_elem], BF16, isOutput=False),
        nc.declare_dram_parameter("Et", [BOND + 1, per_core], BF16,
                                  isOutput=False),
        nc.declare_dram_parameter("idx16", [P, n_chunks * idx_cols], I16,
                                  isOutput=False),
        nc.declare_dram_parameter("W1t", [ATOM, MSG], BF16, isOutput=False),
        nc.declare_dram_parameter("W2tb", [BOND + 1, MSG], BF16,
                                  isOutput=False),
        nc.declare_dram_parameter("out", [per_core, MSG], BF16, isOutput=True),
    )
    with tile.TileContext(nc) as tc:
        _emit_pipeline(nc, tc, n_chunks, low_chunks, k_blk, n_nodes, split,
                       handles, reps=reps, n_queues=n_queues, **probe_kw)
    return nc


# --------------------------------------------------------------------------
# v4: paired gather. Edges are sorted by src; in unique-rank space every
# adjacent pair of sorted edges has rank-delta d in {0, 1}, so ONE 512B
# gather descriptor fetches BOTH edges' V rows from a per-core table
#   Vu2[k] = [V[uniq[k]], V[uniq[k]], V[uniq[k+1]]]        (stride 384)
# (d=0 pairs read columns 0:256, d=1 pairs read columns 128:384). This
# halves the SWDGE descriptor count, which real HW charges per-index.
# Chunks are homogeneous in (d, rank-half); geometry adapts to the input
# and is compiled on first call. Everything else matches v2 (feature-major
# E with ones row, bias via the matmul, bf16 out, slot permutation).

SPLIT_RANK = 21760     # rank base split: both halves < 32768 for any n_u


def _emit_pipeline_v4(nc, tc, groups, k_blk, u_rows, handles, reps=1):
    """groups: list of (n_chunks, base_row, col0) in chunk order."""
    vu_h, e_h, idx_h, w1_h, w2_h, out_h = handles
    chunk = P * k_blk                    # edges per chunk
    n_pairs = chunk // 2                 # gather descriptors per chunk
    idx_cols = n_pairs // 16
    n_chunks = sum(g[0] for g in groups)

    with (
        tc.tile_pool(name="const", bufs=1) as const,
        tc.tile_pool(name="chunkio", bufs=4) as chunkio,
        tc.tile_pool(name="work", bufs=3) as work,
        tc.tile_pool(name="psum", bufs=8, space="PSUM") as psum,
    ):
        w1t = const.tile([ATOM, MSG], BF16)
        nc.sync.dma_start(out=w1t[:], in_=w1_h[:, :])
        w2tb = const.tile([BOND + 1, MSG], BF16)
        nc.sync.dma_start(out=w2tb[:], in_=w2_h[:, :])

        idx_all = const.tile([P, n_chunks * idx_cols], I16)
        pre = min(4, n_chunks)
        nc.sync.dma_start(
            out=idx_all[:, 0:pre * idx_cols], in_=idx_h[:, 0:pre * idx_cols]
        )
        if n_chunks > pre:
            nc.sync.dma_start(
                out=idx_all[:, pre * idx_cols:],
                in_=idx_h[:, pre * idx_cols:],
            )

        chunk_specs = []
        for n_g, base_row, col0 in groups:
            chunk_specs += [(base_row, col0)] * n_g

        def chunk_body():
            for c, (base_row, col0) in enumerate(chunk_specs):
                c0 = c * chunk
                c1 = c0 + chunk

                # pair gather: v_t[:, 0, i] / v_t[:, 1, i] = the two V rows
                # of pair i (512B descriptor, row stride 384 elements)
                v_t = chunkio.tile([P, chunk], BF16, tag="vgat")
                nc.gpsimd.dma_gather(
                    out_ap=v_t[:].rearrange("p (s k) -> p s k", s=2),
                    in_ap=vu_h[base_row:base_row + min(32768, u_rows - base_row),
                               col0:col0 + 2 * ATOM],
                    idxs_ap=idx_all[:, c * idx_cols:(c + 1) * idx_cols],
                    num_idxs=n_pairs,
                    num_idxs_reg=n_pairs,
                    elem_size=2 * ATOM,
                    elem_step=3 * ATOM,
                    transpose=True,
                    single_packet=False,
                )
                e_t = chunkio.tile([BOND + 1, chunk], BF16, tag="ebond")
                nc.scalar.dma_start(out=e_t[:], in_=e_h[:, c0:c1])

                o_grp = work.tile([P, k_blk * MSG], BF16, tag="ogrp")
                half = k_blk // 2
                for j2 in range(0, k_blk, 2):
                    o_ps = psum.tile([P, 2 * MSG], F32, tag="ops")
                    for u in range(2):
                        j = j2 + u
                        # block j < half: A-edges (sub 0); else B-edges
                        sub, jj = (0, j) if j < half else (1, j - half)
                        lhs = v_t[:].rearrange("p (s k) -> p s k", s=2)[
                            :, sub, jj * P:(jj + 1) * P
                        ]
                        nc.tensor.matmul(
                            out=o_ps[:, u * MSG:(u + 1) * MSG],
                            lhsT=lhs,
                            rhs=w1t[:],
                            start=True,
                            stop=False,
                        )
                        nc.tensor.matmul(
                            out=o_ps[:, u * MSG:(u + 1) * MSG],
                            lhsT=e_t[:, j * P:(j + 1) * P],
                            rhs=w2tb[:],
                            start=False,
                            stop=True,
                        )
                    dst = o_grp[:, j2 * MSG:(j2 + 2) * MSG]
                    if (j2 // 2) % 2 == 0:
                        nc.vector.tensor_copy(out=dst, in_=o_ps[:])
                    else:
                        nc.scalar.copy(out=dst, in_=o_ps[:])

                out_view = out_h[c0:c1, :].rearrange("(p k) m -> p k m", p=P)
                nc.sync.dma_start(out=out_view[:, :, :], in_=o_grp[:, :])

        if reps == 1:
            chunk_body()
        else:
            with tc.For_i(0, reps, 1):
                chunk_body()


def build_nc_v4(groups, u_rows, k_blk=K_BLK, reps=1):
    chunk = P * k_blk
    n_chunks = sum(g[0] for g in groups)
    per_core = n_chunks * chunk
    idx_cols = chunk // 2 // 16

    nc = bacc.Bacc()
    handles = (
        nc.declare_dram_parameter("Vu2", [u_rows, 3 * ATOM], BF16,
                                  isOutput=False),
        nc.declare_dram_parameter("Et", [BOND + 1, per_core], BF16,
                                  isOutput=False),
        nc.declare_dram_parameter("idx16", [P, n_chunks * idx_cols], I16,
                                  isOutput=False),
        nc.declare_dram_parameter("W1t", [ATOM, MSG], BF16, isOutput=False),
        nc.declare_dram_parameter("W2tb", [BOND + 1, MSG], BF16,
                                  isOutput=False),
        nc.declare_dram_parameter("out", [per_core, MSG], BF16, isOutput=True),
    )
    with tile.TileContext(nc) as tc:
        _emit_pipeline_v4(nc, tc, groups, k_blk, u_rows, handles, reps=reps)
    return nc


def _make_in_maps_v4(V, E, edge_index, W, b, k_blk=K_BLK, n_cores=N_CORES,
                     per_core_raw=PER_CORE_RAW):
    """Returns (in_maps, placements, groups, u_rows).

    placements[i] is slot_edge: device out-slot -> local edge id (-1 pad).
    """
    V = np.asarray(V, dtype=np.float32)
    E = np.asarray(E, dtype=np.float32)
    W = np.asarray(W, dtype=np.float32)
    b = np.asarray(b, dtype=np.float32)

    chunk = P * k_blk
    n_pairs_c = chunk // 2
    half = k_blk // 2

    src = np.asarray(edge_index[0]).astype(np.int32)
    wt = W.T.astype(NP_BF16)
    w1t = np.ascontiguousarray(wt[:ATOM])
    w2tb = np.ascontiguousarray(
        np.concatenate([wt[ATOM:], b[None, :].astype(NP_BF16)], axis=0)
    )

    # per-core pair groups: (d, rank-half) -> arrays of (first_rank, eA, eB)
    cores = []
    max_nu = 0
    for i in range(n_cores):
        lo = i * per_core_raw
        src_i = src[lo:lo + per_core_raw]
        order = np.argsort(src_i, kind="stable").astype(np.int64)
        uniq, rank_sorted = np.unique(src_i[order], return_inverse=True)
        n_u = len(uniq)
        max_nu = max(max_nu, n_u)
        # pairs of adjacent sorted edges: rank delta is 0 or 1 by construction
        eA = order[0::2]
        eB = order[1::2]
        kA = rank_sorted[0::2].astype(np.int32)
        d = (rank_sorted[1::2] - rank_sorted[0::2]).astype(np.int32)
        assert d.min() >= 0 and d.max() <= 1
        g = {}
        for dd in (0, 1):
            for hh in (0, 1):
                m = (d == dd) & ((kA >= SPLIT_RANK) == bool(hh))
                g[(dd, hh)] = (kA[m], eA[m], eB[m])
        cores.append((g, uniq, n_u))

    # uniform geometry: chunks per group = ceil(max count / pairs-per-chunk)
    group_keys = [(0, 0), (1, 0), (0, 1), (1, 1)]
    n_chunks_g = {
        k: int(np.ceil(max(len(c[0][k][0]) for c in cores) / n_pairs_c))
        for k in group_keys
    }
    u_rows = max_nu + 1
    groups = [
        (n_chunks_g[(dd, hh)], SPLIT_RANK * hh, ATOM * dd)
        for (dd, hh) in group_keys
    ]
    n_chunks = sum(n_chunks_g.values())
    per_core = n_chunks * chunk

    in_maps = []
    placements = []
    for i in range(n_cores):
        g, uniq, n_u = cores[i]
        lo = i * per_core_raw
        e_i = E[lo:lo + per_core_raw]

        # table: [Vu[k], Vu[k], Vu[k+1]] per unique rank k
        vu = V[uniq].astype(NP_BF16)                  # [n_u, 128]
        vu2 = np.zeros((u_rows, 3 * ATOM), NP_BF16)
        vu2[:n_u, 0:ATOM] = vu
        vu2[:n_u, ATOM:2 * ATOM] = vu
        vu2[:n_u - 1, 2 * ATOM:] = vu[1:]
        vu2[n_u - 1, 2 * ATOM:] = vu[n_u - 1]

        # pair-slot layout: group-ordered chunks of n_pairs_c pairs
        pair_rank = np.zeros(n_chunks * n_pairs_c, np.int32)
        pair_eA = np.full(n_chunks * n_pairs_c, -1, np.int64)
        pair_eB = np.full(n_chunks * n_pairs_c, -1, np.int64)
        off = 0
        for (dd, hh) in group_keys:
            kAg, eAg, eBg = g[(dd, hh)]
            n = len(kAg)
            sl = slice(off, off + n)
            pair_rank[sl] = kAg - SPLIT_RANK * hh
            pair_eA[sl] = eAg
            pair_eB[sl] = eBg
            off += n_chunks_g[(dd, hh)] * n_pairs_c

        # wrapped idx16 per chunk
        idx16 = wrap_idx16_chunks(pair_rank.reshape(n_chunks, n_pairs_c))

        # position q = j*128 + p of chunk c:
        #   j < half: edge A of pair j*128+p ... wait: A-block jj uses pairs
        #   jj*128..jj*128+127; B-block likewise. position->edge:
        pos_edge = np.empty((n_chunks, chunk), np.int64)
        pairs_c = np.stack([pair_eA, pair_eB], axis=0).reshape(
            2, n_chunks, n_pairs_c
        )
        pos_edge[:, :half * P] = pairs_c[0]           # A edges, pair order
        pos_edge[:, half * P:] = pairs_c[1]           # B edges, pair order

        safe = np.maximum(pos_edge, 0)
        e_pos = e_i[safe.reshape(-1)].astype(NP_BF16)
        e_pos[pos_edge.reshape(-1) < 0] = 0
        et = np.empty((BOND + 1, per_core), NP_BF16)
        et[:BOND] = e_pos.T
        et[BOND] = 1.0

        # out slot s = c0 + p*k_blk + j holds position j*128+p
        q = np.arange(chunk)
        pos2slot = (q % P) * k_blk + (q // P)
        slot_edge = np.empty(n_chunks * chunk, np.int64)
        slot_view = slot_edge.reshape(n_chunks, chunk)
        for c in range(n_chunks):
            slot_view[c, pos2slot] = pos_edge[c]

        in_maps.append(
            {
                "Vu2": np.ascontiguousarray(vu2),
                "Et": np.ascontiguousarray(et),
                "idx16": idx16,
                "W1t": w1t,
                "W2tb": w2tb,
            }
        )
        placements.append(slot_edge)
    return in_maps, placements, groups, u_rows


def kernel_v4(V, E, edge_index, W, b):
    in_maps, placements, groups, u_rows = _make_in_maps_v4(
        V, E, edge_index, W, b
    )
    key = ("v4", tuple(groups), u_rows)
    if key not in _NC_CACHE:
        nc = build_nc_v4(groups, u_rows)
        nc.finalize()
        _NC_CACHE[key] = nc
    nc = _NC_CACHE[key]
    res = run_bass_kernel_spmd(nc, in_maps, core_ids=list(range(N_CORES)))
    out = np.empty((N_EDGES, MSG), np.float32)
    for i, slot_edge in enumerate(placements):
        dev = np.asarray(res.results[i]["out"])
        valid = slot_edge >= 0
        blk = out[i * PER_CORE_RAW:(i + 1) * PER_CORE_RAW]
        blk[slot_edge[valid]] = dev[valid].astype(np.float32)
    return out


# --------------------------------------------------------------------------
# v3: U-gather design. Host precomputes U = V @ W1.T + b (bf16); the device
# gathers U rows EDGE-major (512B descriptors, int32 offsets, no low/high
# split), computes E @ W2.T into PSUM, and the PSUM evacuation fuses the
# U add and the f32 -> bf16 cast. No transposes anywhere; half the matmuls.

U_CHUNKS = 49                       # 49 * 2048 = 100352 slots (0.35% pad)
U_PER_CORE = U_CHUNKS * CHUNK


def _emit_pipeline_u(nc, tc, n_chunks, k_blk, n_nodes, handles, reps=1,
                     adds="pe", evac="dve-act"):
    from concourse.masks import make_identity

    u_h, e_h, off_h, w2_h, out_h = handles
    chunk = P * k_blk

    with (
        tc.tile_pool(name="const", bufs=1) as const,
        tc.tile_pool(name="chunkio", bufs=4) as chunkio,
        tc.tile_pool(name="work", bufs=3) as work,
        tc.tile_pool(name="psum", bufs=8, space="PSUM") as psum,
    ):
        w2t = const.tile([BOND, MSG], BF16)
        nc.sync.dma_start(out=w2t[:], in_=w2_h[:, :])
        offs_all = const.tile([P, n_chunks * k_blk], mybir.dt.int32)
        nc.sync.dma_start(out=offs_all[:], in_=off_h[:, :])
        ident = None
        if adds == "pe":
            ident = const.tile([P, P], BF16)
            make_identity(nc, ident[:])

        def chunk_body():
            for c in range(n_chunks):
                c0 = c * chunk
                c1 = c0 + chunk

                # edge-major gathered U rows: u_t[p, j, :] = U[src(pos j*128+p)]
                u_t = chunkio.tile([P, k_blk * MSG], BF16, tag="ugat")
                nc.gpsimd.indirect_dma_start(
                    out=u_t[:].rearrange("p (k m) -> p k m", m=MSG),
                    out_offset=None,
                    in_=u_h[:, :],
                    in_offset=bacc.bass.IndirectOffsetOnAxis(
                        ap=offs_all[:, c * k_blk:(c + 1) * k_blk],
                        axis=0,
                    ),
                )
                e_t = chunkio.tile([BOND, chunk], BF16, tag="ebond")
                nc.scalar.dma_start(out=e_t[:], in_=e_h[:, c0:c1])

                o_grp = work.tile([P, k_blk * MSG], BF16, tag="ogrp")
                for j2 in range(0, k_blk, 2):
                    o_ps = psum.tile([P, 2 * MSG], F32, tag="ops")
                    for u in range(2):
                        j = j2 + u
                        nc.tensor.matmul(
                            out=o_ps[:, u * MSG:(u + 1) * MSG],
                            lhsT=e_t[:, j * P:(j + 1) * P],
                            rhs=w2t[:],
                            start=True,
                            stop=(adds != "pe"),
                        )
                        if adds == "pe":
                            # accumulate the gathered U rows into PSUM via an
                            # identity matmul: I.T @ u_t[:, j, :] = u_t block
                            nc.tensor.matmul(
                                out=o_ps[:, u * MSG:(u + 1) * MSG],
                                lhsT=ident[:],
                                rhs=u_t[:, j * MSG:(j + 1) * MSG],
                                start=False,
                                stop=True,
                            )
                    dst = o_grp[:, j2 * MSG:(j2 + 2) * MSG]
                    if adds == "pe":
                        # plain PSUM -> SBUF cast copy
                        if (j2 // 2) % 2 == 0:
                            nc.vector.tensor_copy(out=dst, in_=o_ps[:])
                        else:
                            nc.scalar.copy(out=dst, in_=o_ps[:])
                    else:
                        # fused U add + cast on the DVE
                        nc.vector.tensor_tensor(
                            out=dst, in0=o_ps[:],
                            in1=u_t[:, j2 * MSG:(j2 + 2) * MSG],
                            op=mybir.AluOpType.add,
                        )

                out_view = out_h[c0:c1, :].rearrange("(p k) m -> p k m", p=P)
                nc.sync.dma_start(out=out_view[:, :, :], in_=o_grp[:, :])

        if reps == 1:
            chunk_body()
        else:
            with tc.For_i(0, reps, 1):
                chunk_body()


def build_nc_u(n_chunks=U_CHUNKS, k_blk=K_BLK, n_nodes=N_NODES, reps=1,
               **kw):
    chunk = P * k_blk
    per_core = n_chunks * chunk

    nc = bacc.Bacc()
    handles = (
        nc.declare_dram_parameter("U", [n_nodes, MSG], BF16, isOutput=False),
        nc.declare_dram_parameter("Et", [BOND, per_core], BF16,
                                  isOutput=False),
        nc.declare_dram_parameter("offs", [P, n_chunks * k_blk],
                                  mybir.dt.int32, isOutput=False),
        nc.declare_dram_parameter("W2t", [BOND, MSG], BF16, isOutput=False),
        nc.declare_dram_parameter("out", [per_core, MSG], BF16, isOutput=True),
    )
    with tile.TileContext(nc) as tc:
        _emit_pipeline_u(nc, tc, n_chunks, k_blk, n_nodes, handles, reps=reps,
                         **kw)
    return nc


def _make_in_maps_u(V, E, edge_index, W, b):
    V = np.asarray(V, dtype=np.float32)
    E = np.asarray(E, dtype=np.float32)
    W = np.asarray(W, dtype=np.float32)
    b = np.asarray(b, dtype=np.float32)

    src = np.asarray(edge_index[0]).astype(np.int32)
    U = (V @ W[:, :ATOM].T + b).astype(NP_BF16)    # [50000, 256]
    w2t = np.ascontiguousarray(W[:, ATOM:].T.astype(NP_BF16))

    in_maps = []
    placements = []
    for i in range(N_CORES):
        lo = i * PER_CORE_RAW
        src_i = src[lo:lo + PER_CORE_RAW]
        e_i = E[lo:lo + PER_CORE_RAW]

        slot_edge = np.full(U_PER_CORE, -1, np.int64)
        slot_edge[:PER_CORE_RAW] = np.arange(PER_CORE_RAW)
        pos_edge = slot_edge.reshape(U_CHUNKS, CHUNK)[:, POS2SLOT]
        safe = np.maximum(pos_edge, 0)

        # offsets: offs[p, c*16 + j] = src of edge at position j*128+p
        idx_pos = src_i[safe].astype(np.int32)     # [U_CHUNKS, CHUNK]
        idx_pos[pos_edge < 0] = 0
        offs = np.ascontiguousarray(
            idx_pos.reshape(U_CHUNKS, K_BLK, P).transpose(2, 0, 1)
            .reshape(P, U_CHUNKS * K_BLK)
        )

        e_pos = e_i[safe.reshape(-1)].astype(NP_BF16)
        e_pos[pos_edge.reshape(-1) < 0] = 0
        et = np.ascontiguousarray(e_pos.T)         # [64, U_PER_CORE]

        in_maps.append(
            {"U": U, "Et": et, "offs": offs, "W2t": w2t}
        )
        placements.append(slot_edge)
    return in_maps, placements


def kernel_u(V, E, edge_index, W, b):
    in_maps, placements = _make_in_maps_u(V, E, edge_index, W, b)
    nc = _get_nc("ugather")
    res = run_bass_kernel_spmd(nc, in_maps, core_ids=list(range(N_CORES)))
    out = np.empty((N_EDGES, MSG), np.float32)
    for i, slot_edge in enumerate(placements):
        dev = np.asarray(res.results[i]["out"])
        valid = slot_edge >= 0
        blk = out[i * PER_CORE_RAW:(i + 1) * PER_CORE_RAW]
        blk[slot_edge[valid]] = dev[valid].astype(np.float32)
    return out


def build_nc_null():
    """Null kernel with identical I/O signature — for RPC/transfer calibration."""
    nc = bacc.Bacc()
    nc.declare_dram_parameter("V", [N_NODES, ATOM], BF16, isOutput=False)
    nc.declare_dram_parameter("Et", [BOND + 1, PER_CORE], BF16, isOutput=False)
    nc.declare_dram_parameter("idx16", [P, N_CHUNKS * IDX_COLS], I16,
                              isOutput=False)
    w1_h = nc.declare_dram_parameter("W1t", [ATOM, MSG], BF16, isOutput=False)
    nc.declare_dram_parameter("W2tb", [BOND + 1, MSG], BF16, isOutput=False)
    out_h = nc.declare_dram_parameter("out", [PER_CORE, MSG], BF16,
                                      isOutput=True)
    with tile.TileContext(nc) as tc:
        with tc.tile_pool(name="p", bufs=1) as pool:
            t = pool.tile([P, MSG], BF16)
            nc.sync.dma_start(out=t[:], in_=w1_h[0:P, :])
            nc.sync.dma_start(out=out_h[0:P, :], in_=t[:])
    return nc


_NC_CACHE = {}


def _get_nc(key, **kw):
    if key not in _NC_CACHE:
        builder = {"null": build_nc_null, "ugather": build_nc_u}.get(
            key, build_nc
        )
        nc = builder(**kw)
        nc.finalize()  # run Bacc passes (reg alloc, matmul wait legalization)
        _NC_CACHE[key] = nc
    return _NC_CACHE[key]


def wrap_idx16_chunks(idx_pos):
    """Wrap position-ordered gather indices for dma_gather.

    idx_pos: [n_chunks, chunk] int array, position i of chunk c gathers
    row idx_pos[c, i]. The ucode reads position i from
    [partition i % 16, col i // 16], replicated across the 8 16-partition
    bands; chunks are concatenated along the free dim.
    Returns [128, n_chunks * chunk // 16] int16.
    """
    n_chunks, chunk = idx_pos.shape
    cols = chunk // 16
    a = idx_pos.reshape(n_chunks, cols, 16).transpose(0, 2, 1)  # [nc,16,cols]
    a = np.tile(a, (1, 8, 1))                                   # [nc,128,cols]
    return np.ascontiguousarray(
        a.transpose(1, 0, 2).reshape(P, n_chunks * cols).astype(np.int16)
    )


def _make_in_maps(V, E, edge_index, W, b, k_blk=K_BLK, low_chunks=LOW_CHUNKS,
                  high_chunks=HIGH_CHUNKS):
    V = np.asarray(V, dtype=np.float32)
    E = np.asarray(E, dtype=np.float32)
    W = np.asarray(W, dtype=np.float32)
    b = np.asarray(b, dtype=np.float32)

    chunk = P * k_blk
    n_chunks = low_chunks + high_chunks
    low_slots = low_chunks * chunk
    high_slots = high_chunks * chunk
    per_core = n_chunks * chunk
    q = np.arange(chunk)
    pos2slot = (q % P) * k_blk + (q // P)

    src = np.asarray(edge_index[0]).astype(np.int32)
    v_bf = np.ascontiguousarray(V.astype(NP_BF16))
    wt = W.T.astype(NP_BF16)                       # [192, 256]
    w1t = np.ascontiguousarray(wt[:ATOM])          # [128, 256]
    w2tb = np.ascontiguousarray(
        np.concatenate([wt[ATOM:], b[None, :].astype(NP_BF16)], axis=0)
    )                                              # [65, 256]

    in_maps = []
    placements = []
    for i in range(N_CORES):
        lo = i * PER_CORE_RAW
        src_i = src[lo:lo + PER_CORE_RAW]
        e_i = E[lo:lo + PER_CORE_RAW]

        low_pos = np.flatnonzero(src_i < SPLIT)
        high_pos = np.flatnonzero(src_i >= SPLIT)
        n_low, n_high = len(low_pos), len(high_pos)
        assert n_low <= low_slots and n_high <= high_slots, (n_low, n_high)

        # slot-ordered local edge ids (-1 = padding)
        slot_edge = np.full(per_core, -1, np.int64)
        slot_edge[:n_low] = low_pos
        slot_edge[low_slots:low_slots + n_high] = high_pos

        # position-ordered view: position q of chunk c = slot pos2slot[q]
        pos_edge = slot_edge.reshape(n_chunks, chunk)[:, pos2slot]

        # gather indices (pad -> 0)
        safe_edge = np.maximum(pos_edge, 0)
        idx_pos = src_i[safe_edge].astype(np.int32)
        idx_pos[low_chunks:] -= SPLIT              # high chunks use base SPLIT
        idx_pos[pos_edge < 0] = 0
        idx16 = wrap_idx16_chunks(idx_pos)

        # feature-major E with ones row, position-ordered columns
        e_pos = e_i[safe_edge.reshape(-1)].astype(NP_BF16)   # [per_core, 64]
        e_pos[pos_edge.reshape(-1) < 0] = 0
        et = np.empty((BOND + 1, per_core), NP_BF16)
        et[:BOND] = e_pos.T
        et[BOND] = 1.0

        in_maps.append(
            {
                "V": v_bf,
                "Et": np.ascontiguousarray(et),
                "idx16": idx16,
                "W1t": w1t,
                "W2tb": w2tb,
            }
        )
        placements.append(slot_edge)
    return in_maps, placements


MODE = "v4"            # "v4" = paired gather; "v2" = per-edge gather
N_QUEUES = 1           # multi-queue SWDGE produces wrong results on HW


def kernel(V, E, edge_index, W, b):
    if MODE == "v4":
        return kernel_v4(V, E, edge_index, W, b)
    if MODE == "u":
        return kernel_u(V, E, edge_index, W, b)
    in_maps, placements = _make_in_maps(V, E, edge_index, W, b)
    nc = _get_nc("full", n_queues=N_QUEUES)
    res = run_bass_kernel_spmd(nc, in_maps, core_ids=list(range(N_CORES)))
    out = np.empty((N_EDGES, MSG), np.float32)
    for i, slot_edge in enumerate(placements):
        dev = np.asarray(res.results[i]["out"])
        valid = slot_edge >= 0
        blk = out[i * PER_CORE_RAW:(i + 1) * PER_CORE_RAW]
        blk[slot_edge[valid]] = dev[valid].astype(np.float32)
    return out


def kernel_null(V, E, edge_index, W, b):
    """Calibration: same transfers as kernel(), trivial device work."""
    in_maps, _ = _make_in_maps(V, E, edge_index, W, b)
    nc = _get_nc("null")
    res = run_bass_kernel_spmd(nc, in_maps, core_ids=list(range(N_CORES)))
    return res.results[0]["out"][0, 0]


# revision 19
# speedup vs baseline: 1.1164x; 1.1164x over previous
"""Bond-message embedding kernel for TRN2 (8 NeuronCores, SPMD).

Computes out[e, :] = concat(V[src[e]], E[e]) @ W.T + b for 800k edges.

Sharding: edges split evenly across the 8 cores (data-parallel over the
edge dim); V, W, b replicated; no cross-core comm.

Shipped design ("v4", paired transpose-gather, bf16 end-to-end):
  * All device tensors are bf16 (rel-err budget 2e-2; bf16 keeps it
    ~3e-3). PSUM accumulates in f32.
  * The SWDGE gather on real HW is charged per DESCRIPTOR, so the host
    sorts each core's edges by src node; in unique-rank space every
    adjacent pair of sorted edges has rank-delta d in {0, 1}, and a
    per-core table  Vu2[k] = [V[uniq[k]], V[uniq[k]], V[uniq[k+1]]]
    (row pitch 384 elements) lets ONE 512B transpose-gather descriptor
    fetch BOTH edges of a pair: d=0 pairs read columns 0:256, d=1 pairs
    read columns 128:384 (elem_step=384). This halves descriptor count
    vs a per-edge gather (measured ~2.1x faster end-to-end).
  * dma_gather(transpose=True) writes the rows FEATURE-major so the
    matmuls consume the gather output directly as lhsT - no transposes.
  * E is host-pre-transposed to feature-major [65, n_slots] with a 65th
    row of ones; W2tb = [W.T rows 128:192; b] so the bias falls out of
    the accumulating matmul (exact f32 bias in PSUM, no extra vector op).
  * Chunks (2048 edges = 1024 pairs) are homogeneous in (d, rank-half)
    -- rank halves keep int16 indices valid; the 4-group geometry adapts
    to the input and compiles on first call.
  * Output slots are permuted so each partition stores 16 contiguous
    HBM rows (8KB descriptors); PSUM evacuation (f32 -> bf16 cast)
    alternates between the DVE and Activation engines.
  * The host undoes all permutations and converts bf16 -> f32.
  * Single SWDGE queue: multi-queue (num_swdge_queues > 1) produces
    wrong results on this HW/runtime path (verified empirically).

Per-chunk device work: 1 transpose-gather (1024 x 512B descriptors),
1 E load ([65, 2048] bf16), 16 pairs of accumulating bf16 matmuls
([128|65, 128] x [.., 256] -> PSUM), 8 PSUM->SBUF cast-copies, 1 output
store ([128, 16*256] bf16).
"""

import numpy as np
import ml_dtypes

import concourse.bacc as bacc
import concourse.mybir as mybir
import concourse.tile as tile
from concourse.bass_utils import run_bass_kernel_spmd

F32 = mybir.dt.float32
BF16 = mybir.dt.bfloat16
I16 = mybir.dt.int16
NP_BF16 = ml_dtypes.bfloat16

N_CORES = 8
N_NODES = 50000
ATOM = 128
BOND = 64
MSG = 256
N_EDGES = 800000
SPLIT = 25000          # int16-safe index boundary (both halves < 32768)

PER_CORE_RAW = N_EDGES // N_CORES   # 100000
P = 128
K_BLK = 16                          # 128-edge blocks per chunk
CHUNK = P * K_BLK                   # 2048 edges per chunk
LOW_CHUNKS = 25                     # 51200 low slots (expect ~50000)
HIGH_CHUNKS = 25                    # 51200 high slots (expect ~50000)
N_CHUNKS = LOW_CHUNKS + HIGH_CHUNKS
LOW_SLOTS = LOW_CHUNKS * CHUNK
HIGH_SLOTS = HIGH_CHUNKS * CHUNK
PER_CORE = N_CHUNKS * CHUNK         # 102400 device slots per core

IDX_COLS = CHUNK // 16              # 128 idx columns per chunk

# position q = j*128 + p within a chunk maps to out slot p*K_BLK + j
_Q = np.arange(CHUNK)
POS2SLOT = (_Q % P) * K_BLK + (_Q // P)          # [CHUNK]


def _emit_pipeline(nc, tc, n_chunks, low_chunks, k_blk, n_nodes, split,
                   handles, reps=1, n_queues=1, gather="tr", store=True,
                   eload="act", compute=True, evac="dve-act",
                   single_packet=False, gather_elem=ATOM):
    v_h, e_h, idx_h, w1_h, w2_h, out_h = handles
    chunk = P * k_blk
    idx_cols = chunk // 16
    g_sub = gather_elem // ATOM        # V rows per gather descriptor
    g_nidx = chunk // g_sub            # gather descriptors per chunk

    with (
        tc.tile_pool(name="const", bufs=1) as const,
        tc.tile_pool(name="chunkio", bufs=4) as chunkio,
        tc.tile_pool(name="work", bufs=3) as work,
        tc.tile_pool(name="psum", bufs=8, space="PSUM") as psum,
    ):
        # --- constants -----------------------------------------------------
        w1t = const.tile([ATOM, MSG], BF16)       # W.T rows 0:128 (atom part)
        nc.sync.dma_start(out=w1t[:], in_=w1_h[:, :])
        w2tb = const.tile([BOND + 1, MSG], BF16)  # [W.T rows 128:192; bias]
        nc.sync.dma_start(out=w2tb[:], in_=w2_h[:, :])

        # whole-core gather indices, preloaded in a few big DMAs
        idx_all = const.tile([P, n_chunks * idx_cols], I16)
        pre = min(4, n_chunks)
        nc.sync.dma_start(
            out=idx_all[:, 0:pre * idx_cols], in_=idx_h[:, 0:pre * idx_cols]
        )
        if n_chunks > pre:
            nc.sync.dma_start(
                out=idx_all[:, pre * idx_cols:],
                in_=idx_h[:, pre * idx_cols:],
            )

        o_const = None
        if not compute and store:
            o_const = const.tile([P, k_blk * MSG], BF16)
            nc.vector.memset(o_const[:], 0.25)

        # --- main loop -----------------------------------------------------
        def chunk_body():
            for c in range(n_chunks):
                c0 = c * chunk
                c1 = c0 + chunk
                v_base = (
                    v_h[0:split, :] if c < low_chunks else v_h[split:n_nodes, :]
                )

                # feature-major gathered V: [128 atom features, chunk edges]
                v_t = chunkio.tile([P, chunk], BF16, tag="vgat")
                if gather == "tr":
                    nc.gpsimd.dma_gather(
                        out_ap=v_t[:].rearrange("p (o k) -> p o k", o=g_sub),
                        in_ap=v_base,
                        idxs_ap=idx_all[
                            :, c * idx_cols:c * idx_cols + g_nidx // 16
                        ],
                        num_idxs=g_nidx,
                        num_idxs_reg=g_nidx,
                        elem_size=gather_elem,
                        transpose=True,
                        single_packet=single_packet,
                        queue_num=c % n_queues,
                    )
                elif gather == "flat":
                    nc.gpsimd.dma_gather(
                        out_ap=v_t[:].rearrange("p (k f) -> p k f", f=ATOM),
                        in_ap=v_base,
                        idxs_ap=idx_all[:, c * idx_cols:(c + 1) * idx_cols],
                        num_idxs=chunk,
                        num_idxs_reg=chunk,
                        elem_size=ATOM,
                        transpose=False,
                        single_packet=single_packet,
                        queue_num=c % n_queues,
                    )
                elif gather == "plain":
                    x0 = (c * chunk) % (n_nodes - chunk - 1)
                    nc.sync.dma_start(
                        out=v_t[:],
                        in_=v_h[x0:x0 + chunk, :].rearrange(
                            "(p k) f -> p (k f)", p=P
                        ),
                    )
                # feature-major E (+ ones row): [65, chunk edges]
                e_t = None
                if eload != "off":
                    e_t = chunkio.tile([BOND + 1, chunk], BF16, tag="ebond")
                    eng = nc.scalar if eload == "act" else nc.sync
                    eng.dma_start(out=e_t[:], in_=e_h[:, c0:c1])

                if compute:
                    o_grp = work.tile([P, k_blk * MSG], BF16, tag="ogrp")
                    for j2 in range(0, k_blk, 2):
                        o_ps = psum.tile([P, 2 * MSG], F32, tag="ops")
                        for u in range(2):
                            j = j2 + u
                            nc.tensor.matmul(
                                out=o_ps[:, u * MSG:(u + 1) * MSG],
                                lhsT=v_t[:, j * P:(j + 1) * P],
                                rhs=w1t[:],
                                start=True,
                                stop=(e_t is None),
                            )
                            if e_t is not None:
                                nc.tensor.matmul(
                                    out=o_ps[:, u * MSG:(u + 1) * MSG],
                                    lhsT=e_t[:, j * P:(j + 1) * P],
                                    rhs=w2tb[:],
                                    start=False,
                                    stop=True,
                                )
                        # PSUM -> SBUF evacuation with f32 -> bf16 cast; bias
                        # is in PSUM via the ones-row x bias-row matmul.
                        dst = o_grp[:, j2 * MSG:(j2 + 2) * MSG]
                        half = j2 // 2
                        if evac == "dve-act":
                            use = ("dve", "act")[half % 2]
                        elif evac == "spread":
                            use = ("dve", "act", "dve", "act", "pool",
                                   "dve", "act", "pool")[half % 8]
                        else:
                            use = evac
                        if use == "dve":
                            nc.vector.tensor_copy(out=dst, in_=o_ps[:])
                        elif use == "act":
                            nc.scalar.copy(out=dst, in_=o_ps[:])
                        else:
                            nc.gpsimd.tensor_copy(out=dst, in_=o_ps[:])
                else:
                    o_grp = o_const

                if store:
                    # slot s = c0 + p*k_blk + j holds position j*128+p: each
                    # partition stores k_blk contiguous HBM rows (8KB descs)
                    out_view = out_h[c0:c1, :].rearrange(
                        "(p k) m -> p k m", p=P
                    )
                    nc.sync.dma_start(out=out_view[:, :, :], in_=o_grp[:, :])

        if reps == 1:
            chunk_body()
        else:
            with tc.For_i(0, reps, 1):
                chunk_body()


def build_nc(n_chunks=N_CHUNKS, low_chunks=LOW_CHUNKS, k_blk=K_BLK,
             n_nodes=N_NODES, split=SPLIT, reps=1, n_queues=1, **probe_kw):
    chunk = P * k_blk
    per_core = n_chunks * chunk
    idx_cols = chunk // 16

    nc = bacc.Bacc(num_swdge_queues=n_queues)
    g_elem = probe_kw.get("gather_elem", ATOM)
    handles = (
        nc.declare_dram_parameter("V", [n_nodes, g_elem], BF16, isOutput=False),
        nc.declare_dram_parameter("Et", [BOND + 1, per_core], BF16,
                                  isOutput=False),
        nc.declare_dram_parameter("idx16", [P, n_chunks * idx_cols], I16,
                                  isOutput=False),
        nc.declare_dram_parameter("W1t", [ATOM, MSG], BF16, isOutput=False),
        nc.declare_dram_parameter("W2tb", [BOND + 1, MSG], BF16,
                                  isOutput=False),
        nc.declare_dram_parameter("out", [per_core, MSG], BF16, isOutput=True),
    )
    with tile.TileContext(nc) as tc:
        _emit_pipeline(nc, tc, n_chunks, low_chunks, k_blk, n_nodes, split,
                       handles, reps=reps, n_queues=n_queues, **probe_kw)
    return nc


# --------------------------------------------------------------------------
# v4: paired gather. Edges are sorted by src; in unique-rank space every
# adjacent pair of sorted edges has rank-delta d in {0, 1}, so ONE 512B
# gather descriptor fetches BOTH edges' V rows from a per-core table
#   Vu2[k] = [V[uniq[k]], V[uniq[k]], V[uniq[k+1]]]        (stride 384)
# (d=0 pairs read columns 0:256, d=1 pairs read columns 128:384). This
# halves the SWDGE descriptor count, which real HW charges per-index.
# Chunks are homogeneous in (d, rank-half); geometry adapts to the input
# and is compiled on first call. Everything else matches v2 (feature-major
# E with ones row, bias via the matmul, bf16 out, slot permutation).

SPLIT_RANK = 21760     # rank base split: both halves < 32768 for any n_u


def _emit_pipeline_v4(nc, tc, groups, k_blk, u_rows, handles, reps=1):
    """groups: list of (n_chunks, base_row, col0) in chunk order."""
    vu_h, e_h, idx_h, w1_h, w2_h, out_h = handles
    chunk = P * k_blk                    # edges per chunk
    n_pairs = chunk // 2                 # gather descriptors per chunk
    idx_cols = n_pairs // 16
    n_chunks = sum(g[0] for g in groups)

    with (
        tc.tile_pool(name="const", bufs=1) as const,
        tc.tile_pool(name="chunkio", bufs=6) as chunkio,
        tc.tile_pool(name="work", bufs=3) as work,
        tc.tile_pool(name="psum", bufs=8, space="PSUM") as psum,
    ):
        w1t = const.tile([ATOM, MSG], BF16)
        nc.sync.dma_start(out=w1t[:], in_=w1_h[:, :])
        w2tb = const.tile([BOND + 1, MSG], BF16)
        nc.sync.dma_start(out=w2tb[:], in_=w2_h[:, :])

        idx_all = const.tile([P, n_chunks * idx_cols], I16)
        pre = min(4, n_chunks)
        nc.sync.dma_start(
            out=idx_all[:, 0:pre * idx_cols], in_=idx_h[:, 0:pre * idx_cols]
        )
        if n_chunks > pre:
            nc.sync.dma_start(
                out=idx_all[:, pre * idx_cols:],
                in_=idx_h[:, pre * idx_cols:],
            )

        chunk_specs = []
        for n_g, base_row, col0 in groups:
            chunk_specs += [(base_row, col0)] * n_g

        def chunk_body():
            for c, (base_row, col0) in enumerate(chunk_specs):
                c0 = c * chunk
                c1 = c0 + chunk

                # pair gather: v_t[:, 0, i] / v_t[:, 1, i] = the two V rows
                # of pair i (512B descriptor, row stride 384 elements)
                v_t = chunkio.tile([P, chunk], BF16, tag="vgat")
                nc.gpsimd.dma_gather(
                    out_ap=v_t[:].rearrange("p (s k) -> p s k", s=2),
                    in_ap=vu_h[base_row:base_row + min(32768, u_rows - base_row),
                               col0:col0 + 2 * ATOM],
                    idxs_ap=idx_all[:, c * idx_cols:(c + 1) * idx_cols],
                    num_idxs=n_pairs,
                    num_idxs_reg=n_pairs,
                    elem_size=2 * ATOM,
                    elem_step=3 * ATOM,
                    transpose=True,
                    single_packet=False,
                )
                e_t = chunkio.tile([BOND + 1, chunk], BF16, tag="ebond")
                nc.scalar.dma_start(out=e_t[:], in_=e_h[:, c0:c1])

                o_grp = work.tile([P, k_blk * MSG], BF16, tag="ogrp")
                half = k_blk // 2
                for j2 in range(0, k_blk, 2):
                    o_ps = psum.tile([P, 2 * MSG], F32, tag="ops")
                    for u in range(2):
                        j = j2 + u
                        # block j < half: A-edges (sub 0); else B-edges
                        sub, jj = (0, j) if j < half else (1, j - half)
                        lhs = v_t[:].rearrange("p (s k) -> p s k", s=2)[
                            :, sub, jj * P:(jj + 1) * P
                        ]
                        nc.tensor.matmul(
                            out=o_ps[:, u * MSG:(u + 1) * MSG],
                            lhsT=lhs,
                            rhs=w1t[:],
                            start=True,
                            stop=False,
                        )
                        nc.tensor.matmul(
                            out=o_ps[:, u * MSG:(u + 1) * MSG],
                            lhsT=e_t[:, j * P:(j + 1) * P],
                            rhs=w2tb[:],
                            start=False,
                            stop=True,
                        )
                    dst = o_grp[:, j2 * MSG:(j2 + 2) * MSG]
                    if (j2 // 2) % 2 == 0:
                        nc.vector.tensor_copy(out=dst, in_=o_ps[:])
                    else:
                        nc.scalar.copy(out=dst, in_=o_ps[:])

                out_view = out_h[c0:c1, :].rearrange("(p k) m -> p k m", p=P)
                nc.sync.dma_start(out=out_view[:, :, :], in_=o_grp[:, :])

        if reps == 1:
            chunk_body()
        else:
            with tc.For_i(0, reps, 1):
                chunk_body()


def build_nc_v4(groups, u_rows, k_blk=K_BLK, reps=1):
    chunk = P * k_blk
    n_chunks = sum(g[0] for g in groups)
    per_core = n_chunks * chunk
    idx_cols = chunk // 2 // 16

    nc = bacc.Bacc()
    handles = (
        nc.declare_dram_parameter("Vu2", [u_rows, 3 * ATOM], BF16,
                                  isOutput=False),
        nc.declare_dram_parameter("Et", [BOND + 1, per_core], BF16,
                                  isOutput=False),
        nc.declare_dram_parameter("idx16", [P, n_chunks * idx_cols], I16,
                                  isOutput=False),
        nc.declare_dram_parameter("W1t", [ATOM, MSG], BF16, isOutput=False),
        nc.declare_dram_parameter("W2tb", [BOND + 1, MSG], BF16,
                                  isOutput=False),
        nc.declare_dram_parameter("out", [per_core, MSG], BF16, isOutput=True),
    )
    with tile.TileContext(nc) as tc:
        _emit_pipeline_v4(nc, tc, groups, k_blk, u_rows, handles, reps=reps)
    return nc


def _make_in_maps_v4(V, E, edge_index, W, b, k_blk=K_BLK, n_cores=N_CORES,
                     per_core_raw=PER_CORE_RAW):
    """Returns (in_maps, placements, groups, u_rows).

    placements[i] is slot_edge: device out-slot -> local edge id (-1 pad).
    """
    V = np.asarray(V, dtype=np.float32)
    E = np.asarray(E, dtype=np.float32)
    W = np.asarray(W, dtype=np.float32)
    b = np.asarray(b, dtype=np.float32)

    chunk = P * k_blk
    n_pairs_c = chunk // 2
    half = k_blk // 2

    src = np.asarray(edge_index[0]).astype(np.int32)
    wt = W.T.astype(NP_BF16)
    w1t = np.ascontiguousarray(wt[:ATOM])
    w2tb = np.ascontiguousarray(
        np.concatenate([wt[ATOM:], b[None, :].astype(NP_BF16)], axis=0)
    )

    # per-core pair groups: (d, rank-half) -> arrays of (first_rank, eA, eB)
    cores = []
    max_nu = 0
    for i in range(n_cores):
        lo = i * per_core_raw
        src_i = src[lo:lo + per_core_raw]
        order = np.argsort(src_i, kind="stable").astype(np.int64)
        uniq, rank_sorted = np.unique(src_i[order], return_inverse=True)
        n_u = len(uniq)
        max_nu = max(max_nu, n_u)
        # pairs of adjacent sorted edges: rank delta is 0 or 1 by construction
        eA = order[0::2]
        eB = order[1::2]
        kA = rank_sorted[0::2].astype(np.int32)
        d = (rank_sorted[1::2] - rank_sorted[0::2]).astype(np.int32)
        assert d.min() >= 0 and d.max() <= 1
        g = {}
        for dd in (0, 1):
            for hh in (0, 1):
                m = (d == dd) & ((kA >= SPLIT_RANK) == bool(hh))
                g[(dd, hh)] = (kA[m], eA[m], eB[m])
        cores.append((g, uniq, n_u))

    # uniform geometry: chunks per group = ceil(max count / pairs-per-chunk)
    group_keys = [(0, 0), (1, 0), (0, 1), (1, 1)]
    n_chunks_g = {
        k: int(np.ceil(max(len(c[0][k][0]) for c in cores) / n_pairs_c))
        for k in group_keys
    }
    u_rows = max_nu + 1
    groups = [
        (n_chunks_g[(dd, hh)], SPLIT_RANK * hh, ATOM * dd)
        for (dd, hh) in group_keys
    ]
    n_chunks = sum(n_chunks_g.values())
    per_core = n_chunks * chunk

    in_maps = []
    placements = []
    for i in range(n_cores):
        g, uniq, n_u = cores[i]
        lo = i * per_core_raw
        e_i = E[lo:lo + per_core_raw]

        # table: [Vu[k], Vu[k], Vu[k+1]] per unique rank k
        vu = V[uniq].astype(NP_BF16)                  # [n_u, 128]
        vu2 = np.zeros((u_rows, 3 * ATOM), NP_BF16)
        vu2[:n_u, 0:ATOM] = vu
        vu2[:n_u, ATOM:2 * ATOM] = vu
        vu2[:n_u - 1, 2 * ATOM:] = vu[1:]
        vu2[n_u - 1, 2 * ATOM:] = vu[n_u - 1]

        # pair-slot layout: group-ordered chunks of n_pairs_c pairs
        pair_rank = np.zeros(n_chunks * n_pairs_c, np.int32)
        pair_eA = np.full(n_chunks * n_pairs_c, -1, np.int64)
        pair_eB = np.full(n_chunks * n_pairs_c, -1, np.int64)
        off = 0
        for (dd, hh) in group_keys:
            kAg, eAg, eBg = g[(dd, hh)]
            n = len(kAg)
            sl = slice(off, off + n)
            pair_rank[sl] = kAg - SPLIT_RANK * hh
            pair_eA[sl] = eAg
            pair_eB[sl] = eBg
            off += n_chunks_g[(dd, hh)] * n_pairs_c

        # wrapped idx16 per chunk
        idx16 = wrap_idx16_chunks(pair_rank.reshape(n_chunks, n_pairs_c))

        # position q = j*128 + p of chunk c:
        #   j < half: edge A of pair j*128+p ... wait: A-block jj uses pairs
        #   jj*128..jj*128+127; B-block likewise. position->edge:
        pos_edge = np.empty((n_chunks, chunk), np.int64)
        pairs_c = np.stack([pair_eA, pair_eB], axis=0).reshape(
            2, n_chunks, n_pairs_c
        )
        pos_edge[:, :half * P] = pairs_c[0]           # A edges, pair order
        pos_edge[:, half * P:] = pairs_c[1]           # B edges, pair order

        safe = np.maximum(pos_edge, 0)
        e_pos = e_i[safe.reshape(-1)].astype(NP_BF16)
        e_pos[pos_edge.reshape(-1) < 0] = 0
        et = np.empty((BOND + 1, per_core), NP_BF16)
        et[:BOND] = e_pos.T
        et[BOND] = 1.0

        # out slot s = c0 + p*k_blk + j holds position j*128+p
        q = np.arange(chunk)
        pos2slot = (q % P) * k_blk + (q // P)
        slot_edge = np.empty(n_chunks * chunk, np.int64)
        slot_view = slot_edge.reshape(n_chunks, chunk)
        for c in range(n_chunks):
            slot_view[c, pos2slot] = pos_edge[c]

        in_maps.append(
            {
                "Vu2": np.ascontiguousarray(vu2),
                "Et": np.ascontiguousarray(et),
                "idx16": idx16,
                "W1t": w1t,
                "W2tb": w2tb,
            }
        )
        placements.append(slot_edge)
    return in_maps, placements, groups, u_rows


def kernel_v4(V, E, edge_index, W, b):
    in_maps, placements, groups, u_rows = _make_in_maps_v4(
        V, E, edge_index, W, b
    )
    key = ("v4", tuple(groups), u_rows)
    if key not in _NC_CACHE:
        nc = build_nc_v4(groups, u_rows)
        nc.finalize()
        _NC_CACHE[key] = nc
    nc = _NC_CACHE[key]
    res = run_bass_kernel_spmd(nc, in_maps, core_ids=list(range(N_CORES)))
    out = np.empty((N_EDGES, MSG), np.float32)
    for i, slot_edge in enumerate(placements):
        dev = np.asarray(res.results[i]["out"])
        valid = slot_edge >= 0
        blk = out[i * PER_CORE_RAW:(i + 1) * PER_CORE_RAW]
        blk[slot_edge[valid]] = dev[valid].astype(np.float32)
    return out


# --------------------------------------------------------------------------
# v3: U-gather design. Host precomputes U = V @ W1.T + b (bf16); the device
# gathers U rows EDGE-major (512B descriptors, int32 offsets, no low/high
# split), computes E @ W2.T into PSUM, and the PSUM evacuation fuses the
# U add and the f32 -> bf16 cast. No transposes anywhere; half the matmuls.

U_CHUNKS = 49                       # 49 * 2048 = 100352 slots (0.35% pad)
U_PER_CORE = U_CHUNKS * CHUNK


def _emit_pipeline_u(nc, tc, n_chunks, k_blk, n_nodes, handles, reps=1,
                     adds="pe", evac="dve-act"):
    from concourse.masks import make_identity

    u_h, e_h, off_h, w2_h, out_h = handles
    chunk = P * k_blk

    with (
        tc.tile_pool(name="const", bufs=1) as const,
        tc.tile_pool(name="chunkio", bufs=4) as chunkio,
        tc.tile_pool(name="work", bufs=3) as work,
        tc.tile_pool(name="psum", bufs=8, space="PSUM") as psum,
    ):
        w2t = const.tile([BOND, MSG], BF16)
        nc.sync.dma_start(out=w2t[:], in_=w2_h[:, :])
        offs_all = const.tile([P, n_chunks * k_blk], mybir.dt.int32)
        nc.sync.dma_start(out=offs_all[:], in_=off_h[:, :])
        ident = None
        if adds == "pe":
            ident = const.tile([P, P], BF16)
            make_identity(nc, ident[:])

        def chunk_body():
            for c in range(n_chunks):
                c0 = c * chunk
                c1 = c0 + chunk

                # edge-major gathered U rows: u_t[p, j, :] = U[src(pos j*128+p)]
                u_t = chunkio.tile([P, k_blk * MSG], BF16, tag="ugat")
                nc.gpsimd.indirect_dma_start(
                    out=u_t[:].rearrange("p (k m) -> p k m", m=MSG),
                    out_offset=None,
                    in_=u_h[:, :],
                    in_offset=bacc.bass.IndirectOffsetOnAxis(
                        ap=offs_all[:, c * k_blk:(c + 1) * k_blk],
                        axis=0,
                    ),
                )
                e_t = chunkio.tile([BOND, chunk], BF16, tag="ebond")
                nc.scalar.dma_start(out=e_t[:], in_=e_h[:, c0:c1])

                o_grp = work.tile([P, k_blk * MSG], BF16, tag="ogrp")
                for j2 in range(0, k_blk, 2):
                    o_ps = psum.tile([P, 2 * MSG], F32, tag="ops")
                    for u in range(2):
                        j = j2 + u
                        nc.tensor.matmul(
                            out=o_ps[:, u * MSG:(u + 1) * MSG],
                            lhsT=e_t[:, j * P:(j + 1) * P],
                            rhs=w2t[:],
                            start=True,
                            stop=(adds != "pe"),
                        )
                        if adds == "pe":
                            # accumulate the gathered U rows into PSUM via an
                            # identity matmul: I.T @ u_t[:, j, :] = u_t block
                            nc.tensor.matmul(
                                out=o_ps[:, u * MSG:(u + 1) * MSG],
                                lhsT=ident[:],
                                rhs=u_t[:, j * MSG:(j + 1) * MSG],
                                start=False,
                                stop=True,
                            )
                    dst = o_grp[:, j2 * MSG:(j2 + 2) * MSG]
                    if adds == "pe":
                        # plain PSUM -> SBUF cast copy
                        if (j2 // 2) % 2 == 0:
                            nc.vector.tensor_copy(out=dst, in_=o_ps[:])
                        else:
                            nc.scalar.copy(out=dst, in_=o_ps[:])
                    else:
                        # fused U add + cast on the DVE
                        nc.vector.tensor_tensor(
                            out=dst, in0=o_ps[:],
                            in1=u_t[:, j2 * MSG:(j2 + 2) * MSG],
                            op=mybir.AluOpType.add,
                        )

                out_view = out_h[c0:c1, :].rearrange("(p k) m -> p k m", p=P)
                nc.sync.dma_start(out=out_view[:, :, :], in_=o_grp[:, :])

        if reps == 1:
            chunk_body()
        else:
            with tc.For_i(0, reps, 1):
                chunk_body()


def build_nc_u(n_chunks=U_CHUNKS, k_blk=K_BLK, n_nodes=N_NODES, reps=1,
               **kw):
    chunk = P * k_blk
    per_core = n_chunks * chunk

    nc = bacc.Bacc()
    handles = (
        nc.declare_dram_parameter("U", [n_nodes, MSG], BF16, isOutput=False),
        nc.declare_dram_parameter("Et", [BOND, per_core], BF16,
                                  isOutput=False),
        nc.declare_dram_parameter("offs", [P, n_chunks * k_blk],
                                  mybir.dt.int32, isOutput=False),
        nc.declare_dram_parameter("W2t", [BOND, MSG], BF16, isOutput=False),
        nc.declare_dram_parameter("out", [per_core, MSG], BF16, isOutput=True),
    )
    with tile.TileContext(nc) as tc:
        _emit_pipeline_u(nc, tc, n_chunks, k_blk, n_nodes, handles, reps=reps,
                         **kw)
    return nc


def _make_in_maps_u(V, E, edge_index, W, b):
    V = np.asarray(V, dtype=np.float32)
    E = np.asarray(E, dtype=np.float32)
    W = np.asarray(W, dtype=np.float32)
    b = np.asarray(b, dtype=np.float32)

    src = np.asarray(edge_index[0]).astype(np.int32)
    U = (V @ W[:, :ATOM].T + b).astype(NP_BF16)    # [50000, 256]
    w2t = np.ascontiguousarray(W[:, ATOM:].T.astype(NP_BF16))

    in_maps = []
    placements = []
    for i in range(N_CORES):
        lo = i * PER_CORE_RAW
        src_i = src[lo:lo + PER_CORE_RAW]
        e_i = E[lo:lo + PER_CORE_RAW]

        slot_edge = np.full(U_PER_CORE, -1, np.int64)
        slot_edge[:PER_CORE_RAW] = np.arange(PER_CORE_RAW)
        pos_edge = slot_edge.reshape(U_CHUNKS, CHUNK)[:, POS2SLOT]
        safe = np.maximum(pos_edge, 0)

        # offsets: offs[p, c*16 + j] = src of edge at position j*128+p
        idx_pos = src_i[safe].astype(np.int32)     # [U_CHUNKS, CHUNK]
        idx_pos[pos_edge < 0] = 0
        offs = np.ascontiguousarray(
            idx_pos.reshape(U_CHUNKS, K_BLK, P).transpose(2, 0, 1)
            .reshape(P, U_CHUNKS * K_BLK)
        )

        e_pos = e_i[safe.reshape(-1)].astype(NP_BF16)
        e_pos[pos_edge.reshape(-1) < 0] = 0
        et = np.ascontiguousarray(e_pos.T)         # [64, U_PER_CORE]

        in_maps.append(
            {"U": U, "Et": et, "offs": offs, "W2t": w2t}
        )
        placements.append(slot_edge)
    return in_maps, placements


def kernel_u(V, E, edge_index, W, b):
    in_maps, placements = _make_in_maps_u(V, E, edge_index, W, b)
    nc = _get_nc("ugather")
    res = run_bass_kernel_spmd(nc, in_maps, core_ids=list(range(N_CORES)))
    out = np.empty((N_EDGES, MSG), np.float32)
    for i, slot_edge in enumerate(placements):
        dev = np.asarray(res.results[i]["out"])
        valid = slot_edge >= 0
        blk = out[i * PER_CORE_RAW:(i + 1) * PER_CORE_RAW]
        blk[slot_edge[valid]] = dev[valid].astype(np.float32)
    return out


def build_nc_null():
    """Null kernel with identical I/O signature — for RPC/transfer calibration."""
    nc = bacc.Bacc()
    nc.declare_dram_parameter("V", [N_NODES, ATOM], BF16, isOutput=False)
    nc.declare_dram_parameter("Et", [BOND + 1, PER_CORE], BF16, isOutput=False)
    nc.declare_dram_parameter("idx16", [P, N_CHUNKS * IDX_COLS], I16,
                              isOutput=False)
    w1_h = nc.declare_dram_parameter("W1t", [ATOM, MSG], BF16, isOutput=False)
    nc.declare_dram_parameter("W2tb", [BOND + 1, MSG], BF16, isOutput=False)
    out_h = nc.declare_dram_parameter("out", [PER_CORE, MSG], BF16,
                                      isOutput=True)
    with tile.TileContext(nc) as tc:
        with tc.tile_pool(name="p", bufs=1) as pool:
            t = pool.tile([P, MSG], BF16)
            nc.sync.dma_start(out=t[:], in_=w1_h[0:P, :])
            nc.sync.dma_start(out=out_h[0:P, :], in_=t[:])
    return nc


_NC_CACHE = {}


def _get_nc(key, **kw):
    if key not in _NC_CACHE:
        builder = {"null": build_nc_null, "ugather": build_nc_u}.get(
            key, build_nc
        )
        nc = builder(**kw)
        nc.finalize()  # run Bacc passes (reg alloc, matmul wait legalization)
        _NC_CACHE[key] = nc
    return _NC_CACHE[key]


def wrap_idx16_chunks(idx_pos):
    """Wrap position-ordered gather indices for dma_gather.

    idx_pos: [n_chunks, chunk] int array, position i of chunk c gathers
    row idx_pos[c, i]. The ucode reads position i from
    [partition i % 16, col i // 16], replicated across the 8 16-partition
    bands; chunks are concatenated along the free dim.
    Returns [128, n_chunks * chunk // 16] int16.
    """
    n_chunks, chunk = idx_pos.shape
    cols = chunk // 16
    a = idx_pos.reshape(n_chunks, cols, 16).transpose(0, 2, 1)  # [nc,16,cols]
    a = np.tile(a, (1, 8, 1))                                   # [nc,128,cols]
    return np.ascontiguousarray(
        a.transpose(1, 0, 2).reshape(P, n_chunks * cols).astype(np.int16)
    )


def _make_in_maps(V, E, edge_index, W, b, k_blk=K_BLK, low_chunks=LOW_CHUNKS,
                  high_chunks=HIGH_CHUNKS):
    V = np.asarray(V, dtype=np.float32)
    E = np.asarray(E, dtype=np.float32)
    W = np.asarray(W, dtype=np.float32)
    b = np.asarray(b, dtype=np.float32)

    chunk = P * k_blk
    n_chunks = low_chunks + high_chunks
    low_slots = low_chunks * chunk
    high_slots = high_chunks * chunk
    per_core = n_chunks * chunk
    q = np.arange(chunk)
    pos2slot = (q % P) * k_blk + (q // P)

    src = np.asarray(edge_index[0]).astype(np.int32)
    v_bf = np.ascontiguousarray(V.astype(NP_BF16))
    wt = W.T.astype(NP_BF16)                       # [192, 256]
    w1t = np.ascontiguousarray(wt[:ATOM])          # [128, 256]
    w2tb = np.ascontiguousarray(
        np.concatenate([wt[ATOM:], b[None, :].astype(NP_BF16)], axis=0)
    )                                              # [65, 256]

    in_maps = []
    placements = []
    for i in range(N_CORES):
        lo = i * PER_CORE_RAW
        src_i = src[lo:lo + PER_CORE_RAW]
        e_i = E[lo:lo + PER_CORE_RAW]

        low_pos = np.flatnonzero(src_i < SPLIT)
        high_pos = np.flatnonzero(src_i >= SPLIT)
        n_low, n_high = len(low_pos), len(high_pos)
        assert n_low <= low_slots and n_high <= high_slots, (n_low, n_high)

        # slot-ordered local edge ids (-1 = padding)
        slot_edge = np.full(per_core, -1, np.int64)
        slot_edge[:n_low] = low_pos
        slot_edge[low_slots:low_slots + n_high] = high_pos

        # position-ordered view: position q of chunk c = slot pos2slot[q]
        pos_edge = slot_edge.reshape(n_chunks, chunk)[:, pos2slot]

        # gather indices (pad -> 0)
        safe_edge = np.maximum(pos_edge, 0)
        idx_pos = src_i[safe_edge].astype(np.int32)
        idx_pos[low_chunks:] -= SPLIT              # high chunks use base SPLIT
        idx_pos[pos_edge < 0] = 0
        idx16 = wrap_idx16_chunks(idx_pos)

        # feature-major E with ones row, position-ordered columns
        e_pos = e_i[safe_edge.reshape(-1)].astype(NP_BF16)   # [per_core, 64]
        e_pos[pos_edge.reshape(-1) < 0] = 0
        et = np.empty((BOND + 1, per_core), NP_BF16)
        et[:BOND] = e_pos.T
        et[BOND] = 1.0

        in_maps.append(
            {
                "V": v_bf,
                "Et": np.ascontiguousarray(et),
                "idx16": idx16,
                "W1t": w1t,
                "W2tb": w2tb,
            }
        )
        placements.append(slot_edge)
    return in_maps, placements


MODE = "v4"            # "v4" = paired gather; "v2" = per-edge gather
N_QUEUES = 1           # multi-queue SWDGE produces wrong results on HW


def kernel(V, E, edge_index, W, b):
    if MODE == "v4":
        return kernel_v4(V, E, edge_index, W, b)
    if MODE == "u":
        return kernel_u(V, E, edge_index, W, b)
    in_maps, placements = _make_in_maps(V, E, edge_index, W, b)
    nc = _get_nc("full", n_queues=N_QUEUES)
    res = run_bass_kernel_spmd(nc, in_maps, core_ids=list(range(N_CORES)))
    out = np.empty((N_EDGES, MSG), np.float32)
    for i, slot_edge in enumerate(placements):
        dev = np.asarray(res.results[i]["out"])
        valid = slot_edge >= 0
        blk = out[i * PER_CORE_RAW:(i + 1) * PER_CORE_RAW]
        blk[slot_edge[valid]] = dev[valid].astype(np.float32)
    return out


def kernel_null(V, E, edge_index, W, b):
    """Calibration: same transfers as kernel(), trivial device work."""
    in_maps, _ = _make_in_maps(V, E, edge_index, W, b)
    nc = _get_nc("null")
    res = run_bass_kernel_spmd(nc, in_maps, core_ids=list(range(N_CORES)))
    return res.results[0]["out"][0, 0]
